# revision 7
# baseline (speedup 1.0000x reference)
"""Trainium2 Bass kernel for CustomMultiHeadAttention (relative position
bias via Music-Transformer skew, causal mask).

Sharding: data-parallel over batch (B=8, one batch element per core; no
collectives).

Staging strategy: inputs are converted to fp16 and pre-transposed on the
host (QT/KT/VT [D,S], W*T [D,D] = W.T, ErT [DK,S]) so the kernel does
zero on-chip transposition of inputs/weights and all matmuls stream at
1 cycle/row; the output is produced transposed [D,S] and un-transposed on
the host.

Per-core structure (software-pipelined emission; stage A runs two pairs
ahead of stage B, with the v-projection filling the pipeline-fill slot):
  - Batched 3D-AP DMA loads of fp16 transposed weights/activations.
  - q^T/k^T projections -> fp16 [j, s] tiles; v projection -> natural
    fp16 [t, 65-interleaved head dims + ones col] (the ones column makes
    the AV matmul emit the softmax denominator Z as a 65th row).
  - Stage A per head-pair: QEr strips -> PSUM -> fp16 SBUF -> DRAM
    scratch (strips packed in width-paired groups; even-strip pad columns
    pre-set to -3e4 so diagonal-tile junk reads arrive pre-masked);
    diagonal-AP read-back (the skew) casting fp16 -> f32r, two strips +
    both heads per DMA.
  - Stage B per head-pair: QK fp16 (heads on disjoint PE row-halves) +
    f32r transpose-accumulate of Srel into the same PSUM + mask matmul on
    odd-diagonal blocks; Exp (scale=1/8) -> packed-causal A^T fp16; AV in
    two s-halves; normalize by 1/Z via reciprocal + ones-broadcast matmul.
  - Output projection (fp16) + bias -> out^T f32, interleaved into the
    last pairs' idle PE slots.
"""

import os

import numpy as np

import concourse.bass as bass
import concourse.tile as tile
from concourse import bacc, mybir
from concourse.bass import AP
from concourse.bass_utils import run_bass_kernel_spmd
from concourse.masks import make_identity

N_CORES = 8
B, S, D, H, DK = 8, 1024, 768, 12, 64
NT = S // 128          # 8 s-tiles
NI = D // 128          # 6 d-blocks
f32 = mybir.dt.float32
f16 = mybir.dt.float16
f32r = mybir.dt.float32r
# QEr scratch packing: strips si (width 128*(si+1)) stored in PAIRS at the
# pair's max width so a single 3D-AP DMA can read two strips at once.
PITCH = [128 * (2 * (si // 2) + 2) for si in range(NT)]   # per-strip pitch
G = [0]
for _g in range(NT // 2):
    G.append(G[-1] + 2 * PITCH[2 * _g])                   # group offsets
OFF2 = [G[si // 2] + (si % 2) * PITCH[si] for si in range(NT)]
RW = G[-1]             # 5120 scratch row width
# A^T packed-causal: t-block ti (s-width 1024-128*ti) at col OT[ti]
OT = [0]
for _ti in range(NT):
    OT.append(OT[-1] + S - 128 * _ti)
AW = OT[-1]            # 4608

HEAD_REG = 128 * RW          # per-head scratch region (elements)
PAIR_REG = 2 * HEAD_REG      # per-pair


def build_nc():
    nc = bacc.Bacc("TRN2", target_bir_lowering=False, debug=False,
                   num_devices=N_CORES)

    QT = nc.dram_tensor("QT", [D, S], f16, kind="ExternalInput")
    KT = nc.dram_tensor("KT", [D, S], f16, kind="ExternalInput")
    VT = nc.dram_tensor("VT", [D, S], f16, kind="ExternalInput")
    WqT = nc.dram_tensor("WqT", [D, D], f16, kind="ExternalInput")
    WkT = nc.dram_tensor("WkT", [D, D], f16, kind="ExternalInput")
    WvT = nc.dram_tensor("WvT", [D, D], f16, kind="ExternalInput")
    WoT = nc.dram_tensor("WoT", [D, D], f16, kind="ExternalInput")
    bq = nc.dram_tensor("bq", [D], f32, kind="ExternalInput")
    bk = nc.dram_tensor("bk", [D], f32, kind="ExternalInput")
    bv = nc.dram_tensor("bv", [D], f32, kind="ExternalInput")
    bo = nc.dram_tensor("bo", [D], f32, kind="ExternalInput")
    ErT = nc.dram_tensor("ErT", [DK, S], f16, kind="ExternalInput")
    out = nc.dram_tensor("out", [D, S], f32, kind="ExternalOutput")

    qer_dram = nc.dram_tensor("qer_scratch", [3 * PAIR_REG], f16)

    tensors = dict(QT=QT, KT=KT, VT=VT, WqT=WqT, WkT=WkT, WvT=WvT, WoT=WoT,
                   bq=bq, bk=bk, bv=bv, bo=bo, ErT=ErT, out=out,
                   qer_dram=qer_dram)
    unroll = int(os.environ.get("BASS_UNROLL", "1"))
    with tile.TileContext(nc) as tc:
        for _ in range(unroll):
            _build_body(nc, tc, tensors)
    nc.compile()
    return nc


def _load_blocked_half(nc, dst_tile, src_dram, nrow, ncol, nblk, halves, hh):
    hw = ncol // halves
    dst3 = dst_tile[:, :].rearrange(
        "p (b c) -> p b c", b=nblk)[:, :, hh * hw:(hh + 1) * hw]
    src = AP(tensor=src_dram, offset=hh * hw,
             ap=[[ncol, 128], [128 * ncol, nblk], [1, hw]])
    nc.sync.dma_start(out=dst3, in_=src)


def _load_blocked(nc, dst_tile, src_dram, nrow, ncol, nblk, halves=1):
    """src [nblk*128, ncol] -> dst [128, nblk*ncol] where dst block ib holds
    src rows [128*ib, 128*ib+128). `halves` splits along ncol for earlier
    consumption."""
    for hh in range(halves):
        _load_blocked_half(nc, dst_tile, src_dram, nrow, ncol, nblk, halves, hh)


def _build_body(nc, tc, t):
    QT, KT, VT = t["QT"], t["KT"], t["VT"]
    WqT, WkT, WvT, WoT = t["WqT"], t["WkT"], t["WvT"], t["WoT"]
    bq, bk, bv, bo = t["bq"], t["bk"], t["bv"], t["bo"]
    ErT, out, qer_dram = t["ErT"], t["out"], t["qer_dram"]

    from contextlib import ExitStack
    with ExitStack() as ctx:
        persist = ctx.enter_context(tc.tile_pool(name="persist", bufs=1))

        ident = persist.tile([128, 128], f32, tag="ident")
        make_identity(nc, ident[:])
        ident16 = persist.tile([128, 128], f16, tag="ident16")
        nc.vector.tensor_copy(ident16[:], ident[:])
        ident32r = persist.tile([128, 128], f32r, tag="ident32r")
        nc.vector.tensor_copy(ident32r[:], ident[:])
        ones16 = persist.tile([1, 64], f16, tag="ones16")
        nc.vector.memset(ones16[:], 1.0)
        # causal mask tile for diagonal blocks, [t, s]: -3e4 where t > s
        mask16 = persist.tile([128, 128], f16, tag="mask16")
        nc.vector.memset(mask16[:], 0.0)
        nc.gpsimd.affine_select(
            out=mask16[:], in_=mask16[:], pattern=[[1, 128]],
            compare_op=mybir.AluOpType.is_ge, fill=-30000.0, base=0,
            channel_multiplier=-1)

        # ---- biases (small, needed by first projection evictions) ----
        bq_col = persist.tile([128, NI], f32, tag="bq_col")
        nc.sync.dma_start(out=bq_col[:],
                          in_=AP(tensor=bq, offset=0, ap=[[1, 128], [128, NI]]))
        bk_col = persist.tile([128, NI], f32, tag="bk_col")
        nc.sync.dma_start(out=bk_col[:],
                          in_=AP(tensor=bk, offset=0, ap=[[1, 128], [128, NI]]))

        # ---- persistent tiles ----
        qT6 = [persist.tile([128, S], f16, tag=f"qT{i}", name=f"qT{i}")
               for i in range(NI)]
        kT6 = [persist.tile([128, S], f16, tag=f"kT{i}", name=f"kT{i}")
               for i in range(NI)]
        v16 = [persist.tile([128, H * 65], f16, tag=f"v16{i}", name=f"v16{i}")
               for i in range(NT)]
        woT = persist.tile([128, NI * D], f16, tag="woT")
        attn_outT = [persist.tile([128, S], f16, tag=f"aoT{i}", name=f"aoT{i}")
                     for i in range(NI)]
        erT2 = persist.tile([128, S], f16, tag="erT2")
        bv_row = persist.tile([128, D], f16, tag="bv_row")
        bo_col = persist.tile([128, NI], f32, tag="bo_col")

        # ---- projections + attention: software-pipelined emission ----
        with tc.tile_pool(name="stage_w", bufs=1) as wstg, \
             tc.tile_pool(name="stage_x", bufs=1) as xstg, \
             tc.tile_pool(name="ps_misc", bufs=3, space="PSUM") as ps_misc, \
             tc.tile_pool(name="work", bufs=2) as work, \
             tc.tile_pool(name="srelp", bufs=2) as srelp, \
             tc.tile_pool(name="ps_qk", bufs=3, space="PSUM") as ps_qk, \
             tc.tile_pool(name="ps_av", bufs=2, space="PSUM") as ps_av:
            # q^T / k^T : psum[j, s-chunk] = sum_ib WT[ib-block] @ XT[ib-block]
            def emit_qk_proj(X, W, bias_col, xT_out, wname):
                wtile = wstg.tile([128, NI * D], f16, tag="w", name=wname)
                xtile = xstg.tile([128, NI * S], f16, tag="x", name=f"x{wname}")
                # interleave halves so the first (w, x) pair lands ASAP
                for hh in range(2):
                    _load_blocked_half(nc, wtile, W, D, D, NI, 2, hh)
                    _load_blocked_half(nc, xtile, X, D, S, NI, 2, hh)
                if wname == "wq":
                    # ErT duplicated into both partition halves (needed by
                    # the first attention pair, right after the q projection)
                    nc.sync.dma_start(out=erT2[0:DK, :], in_=ErT.ap()[:, :])
                    nc.sync.dma_start(out=erT2[DK:128, :], in_=ErT.ap()[:, :])
                for sh in range(2):
                    for jt in range(NI):
                        p = ps_misc.tile([128, 512], f32, tag="misc")
                        for ib in range(NI):
                            nc.tensor.matmul(
                                p[:],
                                wtile[:, D * ib + 128 * jt:D * ib + 128 * jt + 128],
                                xtile[:, S * ib + 512 * sh:S * ib + 512 * sh + 512],
                                start=(ib == 0), stop=(ib == NI - 1),
                            )
                        with nc.allow_low_precision(reason="fp16 activations"):
                            nc.vector.tensor_scalar_add(
                                xT_out[jt][:, sh * 512:(sh + 1) * 512], p[:],
                                bias_col[:, jt:jt + 1],
                            )

            def emit_v_proj():
                # v natural: psum[t, j] = sum_ib XvT[ib][:, t-block] @ WvT[ib]
                nc.gpsimd.dma_start(
                    out=bv_row[:],
                    in_=AP(tensor=bv, offset=0, ap=[[0, 128], [1, D]]))
                for tt in range(NT):
                    ones_col = v16[tt][:, :].rearrange(
                        "p (a b) -> p a b", b=65)[:, :, 64:65]
                    nc.vector.memset(ones_col, 1.0)
                wtile = wstg.tile([128, NI * D], f16, tag="w", name="wv")
                _load_blocked(nc, wtile, WvT, D, D, NI)
                xtile = xstg.tile([128, NI * S], f16, tag="x", name="xwv")
                _load_blocked(nc, xtile, VT, D, S, NI)
                # deferred loads needed only by the output projection
                _load_blocked(nc, woT, WoT, D, D, NI)
                nc.sync.dma_start(
                    out=bo_col[:],
                    in_=AP(tensor=bo, offset=0, ap=[[1, 128], [128, NI]]))
                for tt in range(NT):
                    for js, w in ((0, 512), (512, 256)):
                        p = ps_misc.tile([128, 512], f32, tag="misc")
                        for ib in range(NI):
                            nc.tensor.matmul(
                                p[:, :w],
                                xtile[:, S * ib + 128 * tt:S * ib + 128 * tt + 128],
                                wtile[:, D * ib + js:D * ib + js + w],
                                start=(ib == 0), stop=(ib == NI - 1),
                            )
                        hh0 = js // 64
                        nh = w // 64
                        dst3 = v16[tt][:, :].rearrange(
                            "p (a b) -> p a b", b=65)[:, hh0:hh0 + nh, 0:64]
                        with nc.allow_low_precision(reason="fp16 activations"):
                            nc.vector.tensor_add(
                                dst3, p[:, :w].rearrange("p (a b) -> p a b", b=64),
                                bv_row[:, js:js + w].rearrange("p (a b) -> p a b", b=64),
                            )

            pair_state = {}

            def stage_a(hp):
                # QEr strips -> fp16 -> DRAM scratch -> skewed read-back
                jb = hp
                pair_base = (hp % 3) * PAIR_REG
                ctxs = [dict(h=2 * hp + idx, jr=64 * idx, idx=idx)
                        for idx in range(2)]
                for c in ctxs:
                    c["srel_out"] = work.tile(
                        [128, RW], f16, tag="srel_out",
                        name=f"srel_out{c['h']}", bufs=2)
                    # pad columns of even strips hold -3e4: the diagonal
                    # tile's above-diagonal junk reads land exactly there,
                    # so the srel accumulate applies the causal mask for
                    # free (odd strips still need the mask matmul)
                    for si in range(0, NT, 2):
                        wv = 128 * (si + 1)
                        nc.vector.memset(
                            c["srel_out"][:, OFF2[si] + wv:OFF2[si] + PITCH[si]],
                            -30000.0)
                for si in range(NT):
                    Wcw = 128 * (si + 1)
                    e0 = S - Wcw
                    for cs in range(0, Wcw, 512):
                        w = min(512, Wcw - cs)
                        for c in ctxs:
                            jr = c["jr"]
                            qTs = qT6[jb][jr:jr + 64, si * 128:(si + 1) * 128]
                            p = ps_misc.tile([128, 512], f32, tag="misc")
                            nc.tensor.matmul(
                                p[:, :w], qTs,
                                erT2[jr:jr + 64, e0 + cs:e0 + cs + w],
                                start=True, stop=True)
                            dstp = c["srel_out"][:, OFF2[si] + cs:OFF2[si] + cs + w]
                            with nc.allow_low_precision(reason="fp16 srel"):
                                if (si + c["idx"]) % 2 == 0:
                                    nc.vector.tensor_copy(dstp, p[:, :w])
                                else:
                                    nc.scalar.copy(dstp, p[:, :w])
                for c in ctxs:
                    dst = AP(tensor=qer_dram,
                             offset=pair_base + c["idx"] * HEAD_REG,
                             ap=[[RW, 128], [1, RW]])
                    nc.sync.dma_start(out=dst, in_=c["srel_out"][:])

                # --- skew reads (strip-pair-batched, f16 -> f32r cast) ---
                for c in ctxs:
                    c["srel32"] = srelp.tile([128, RW], f32r, tag="srel32",
                                             name=f"srel32_{c['h']}", bufs=3)
                    head_base = pair_base + c["idx"] * HEAD_REG
                    for g in range(NT // 2):
                        pit = PITCH[2 * g]
                        skew = AP(tensor=qer_dram,
                                  offset=head_base + G[g] + 127,
                                  ap=[[RW - 1, 128], [pit, 2], [1, pit]])
                        dst3 = c["srel32"][:, :].rearrange(
                            "p (x) -> p x")[:, G[g]:G[g] + 2 * pit].rearrange(
                            "p (a b) -> p a b", b=pit)
                        nc.gpsimd.dma_start(out=dst3, in_=skew)
                pair_state[hp] = ctxs

            def stage_b(hp):
                # QK + srel-transpose-accumulate + causal mask + exp + AV
                jb = hp
                ctxs = pair_state.pop(hp)
                for c in ctxs:
                    c["A16T"] = work.tile([128, AW], f16, tag="A16T",
                                          name=f"A16T_{c['h']}", bufs=2)
                for ti in range(NT):
                    s0 = 128 * ti
                    w = S - s0
                    for cs in range(0, w, 512):
                        cw = min(512, w - cs)
                        pqks = []
                        for c in ctxs:
                            jr = c["jr"]
                            pqk = ps_qk.tile([128, 512], f32, tag="qk",
                                             name=f"pqk{c['idx']}")
                            nc.tensor.matmul(
                                pqk[:, :cw],
                                kT6[jb][jr:jr + 64, ti * 128:(ti + 1) * 128],
                                qT6[jb][jr:jr + 64, s0 + cs:s0 + cs + cw],
                                start=True, stop=False)
                            pqks.append(pqk)
                        for c, pqk in zip(ctxs, pqks):
                            nch = cw // 128
                            has_mask = (cs == 0 and ti % 2 == 1)
                            for k in range(nch):
                                sic = ti + (cs + k * 128) // 128
                                last = (k == nch - 1) and not has_mask
                                nc.tensor.matmul(
                                    pqk[:, k * 128:(k + 1) * 128].bitcast(f32r),
                                    c["srel32"][:, OFF2[sic] + 128 * ti:
                                                OFF2[sic] + 128 * ti + 128],
                                    ident32r[:],
                                    is_transpose=True,
                                    start=False, stop=last)
                            if cs == 0 and ti % 2 == 1:
                                # causal mask on the diagonal block (odd
                                # strips have no pad region in the scratch)
                                nc.tensor.matmul(
                                    pqk[:, 0:128], ident16[:], mask16[:],
                                    start=False, stop=True)
                            nc.scalar.activation(
                                c["A16T"][:, OT[ti] + cs:OT[ti] + cs + cw],
                                pqk[:, :cw],
                                mybir.ActivationFunctionType.Exp, scale=0.125)

                # --- AV in two s-halves + normalize ---
                for c in ctxs:
                    h, jr, idx = c["h"], c["jr"], c["idx"]
                    for sh in range(2):
                        slo = 512 * sh
                        pav = ps_av.tile([65, 512], f32, tag="av",
                                         name=f"pav{idx}_{sh}")
                        tis = [ti for ti in range(NT) if 128 * ti < slo + 512]
                        for ti in tis:
                            lo = max(slo, 128 * ti)
                            a0 = OT[ti] + lo - 128 * ti
                            nc.tensor.matmul(
                                pav[:, lo - slo:512],
                                v16[ti][:, h * 65:(h + 1) * 65],
                                c["A16T"][:, a0:a0 + (slo + 512 - lo)],
                                start=(ti == tis[0]), stop=(ti == tis[-1]))
                        rZ = work.tile([1, 512], f16, tag="rZ",
                                       name=f"rZ{h}_{sh}")
                        with nc.allow_low_precision(reason="fp16 softmax Z"):
                            nc.vector.reciprocal(rZ[:], pav[64:65, :])
                        prz = ps_av.tile([64, 512], f32, tag="av",
                                         name=f"prz{idx}")
                        nc.tensor.matmul(prz[:], ones16[:], rZ[:],
                                         start=True, stop=True)
                        rzb = work.tile([64, 512], f16, tag="rzb",
                                        name=f"rzb{idx}_{sh}", bufs=2)
                        with nc.allow_low_precision(reason="fp16 attn out"):
                            if (idx + sh) % 2 == 0:
                                nc.vector.tensor_copy(rzb[:], prz[:])
                            else:
                                nc.scalar.copy(rzb[:], prz[:])
                            if idx == 0:
                                nc.vector.tensor_mul(
                                    attn_outT[jb][0:64, slo:slo + 512],
                                    pav[0:64, :], rzb[:])
                            else:
                                odd_tmp = work.tile([64, 512], f16,
                                                    tag="odd_tmp",
                                                    name=f"ot{h}_{sh}", bufs=2)
                                nc.vector.tensor_mul(
                                    odd_tmp[:], pav[0:64, :], rzb[:])
                                nc.sync.dma_start(
                                    out=attn_outT[jb][64:128, slo:slo + 512],
                                    in_=odd_tmp[:])

            # --- pipelined emission: stage A runs two pairs ahead ---
            NP = H // 2
            emit_qk_proj(QT, WqT, bq_col, qT6, "wq")
            stage_a(0)
            emit_qk_proj(KT, WkT, bk_col, kT6, "wk")
            stage_a(1)
            emit_v_proj()
            for hp in range(NP):
                stage_b(hp)
                if hp + 2 < NP:
                    stage_a(hp + 2)

            # ---- output projection (stored transposed; host un-transposes)
            # reuses the attention pools so it can interleave with the
            # final pairs instead of waiting for all PSUM banks to free
            for sh in range(2):
                for jt in range(NI):
                    p = ps_qk.tile([128, 512], f32, tag="qk", name="po")
                    for ib in range(NI):
                        nc.tensor.matmul(
                            p[:],
                            woT[:, D * ib + 128 * jt:D * ib + 128 * jt + 128],
                            attn_outT[ib][:, sh * 512:(sh + 1) * 512],
                            start=(ib == 0), stop=(ib == NI - 1))
                    osb = work.tile([128, 512], f32, tag="osb", bufs=2)
                    nc.vector.tensor_scalar_add(osb[:], p[:],
                                                bo_col[:, jt:jt + 1])
                    nc.sync.dma_start(
                        out=out.ap()[jt * 128:(jt + 1) * 128,
                                     sh * 512:(sh + 1) * 512],
                        in_=osb[:])


_NC = None


def make_in_maps(**inputs):
    f = np.float16
    Q = np.asarray(inputs["Q"], dtype=np.float32)
    K = np.asarray(inputs["K"], dtype=np.float32)
    V = np.asarray(inputs["V"], dtype=np.float32)
    shared = {
        "WqT": np.ascontiguousarray(np.asarray(inputs["Wq"]).T.astype(f)),
        "WkT": np.ascontiguousarray(np.asarray(inputs["Wk"]).T.astype(f)),
        "WvT": np.ascontiguousarray(np.asarray(inputs["Wv"]).T.astype(f)),
        "WoT": np.ascontiguousarray(np.asarray(inputs["Wo"]).T.astype(f)),
        "ErT": np.ascontiguousarray(np.asarray(inputs["Er"]).T.astype(f)),
        "bq": np.ascontiguousarray(np.asarray(inputs["bq"], dtype=np.float32)),
        "bk": np.ascontiguousarray(np.asarray(inputs["bk"], dtype=np.float32)),
        "bv": np.ascontiguousarray(np.asarray(inputs["bv"], dtype=np.float32)),
        "bo": np.ascontiguousarray(np.asarray(inputs["bo"], dtype=np.float32)),
    }
    return [
        {
            "QT": np.ascontiguousarray(Q[c].T.astype(f)),
            "KT": np.ascontiguousarray(K[c].T.astype(f)),
            "VT": np.ascontiguousarray(V[c].T.astype(f)),
            **shared,
        }
        for c in range(N_CORES)
    ]


def unshard(shards):
    # kernel stores out^T [D, S]; un-transpose host-side
    return np.stack([np.ascontiguousarray(shards[c].T) for c in range(N_CORES)],
                    axis=0)


def kernel(**inputs):
    global _NC
    if _NC is None:
        _NC = build_nc()
    in_maps = make_in_maps(**inputs)
    global _last_in_maps
    _last_in_maps = in_maps
    res = run_bass_kernel_spmd(_NC, in_maps, list(range(N_CORES)))
    return unshard([res.results[c]["out"] for c in range(N_CORES)])


# revision 10
# speedup vs baseline: 1.0754x; 1.0754x over previous
"""Trainium2 Bass kernel for CustomMultiHeadAttention (relative position
bias via Music-Transformer skew, causal mask).

Sharding: data-parallel over batch (B=8, one batch element per core; no
collectives).

Staging strategy: inputs are converted to fp16 and pre-transposed on the
host (QT/KT/VT [D,S], W*T [D,D] = W.T, ErT [DK,S]) so the kernel does
zero on-chip transposition of inputs/weights and all matmuls stream at
1 cycle/row; the output is produced transposed [D,S] and un-transposed on
the host.

Per-core structure (software-pipelined emission; stage A runs two pairs
ahead of stage B, with the v-projection filling the pipeline-fill slot):
  - Batched 3D-AP DMA loads of fp16 transposed weights/activations.
  - q^T/k^T projections -> fp16 [j, s] tiles; v projection -> natural
    fp16 [t, 65-interleaved head dims + ones col] (the ones column makes
    the AV matmul emit the softmax denominator Z as a 65th row).
  - Stage A per head-pair: QEr strips -> PSUM -> fp16 SBUF -> DRAM
    scratch (strips packed in width-paired groups; even-strip pad columns
    pre-set to -3e4 so diagonal-tile junk reads arrive pre-masked);
    diagonal-AP read-back (the skew) casting fp16 -> f32r, two strips +
    both heads per DMA.
  - Stage B per head-pair: QK fp16 (heads on disjoint PE row-halves) +
    f32r transpose-accumulate of Srel into the same PSUM + mask matmul on
    odd-diagonal blocks; Exp (scale=1/8) -> packed-causal A^T fp16; AV in
    two s-halves; normalize by 1/Z via reciprocal + ones-broadcast matmul.
  - Output projection (fp16) + bias -> out^T f32, interleaved into the
    last pairs' idle PE slots.
"""

import os

import numpy as np

import concourse.bass as bass
import concourse.tile as tile
from concourse import bacc, mybir
from concourse.bass import AP
from concourse.bass_utils import run_bass_kernel_spmd
from concourse.masks import make_identity

N_CORES = 8
B, S, D, H, DK = 8, 1024, 768, 12, 64
NT = S // 128          # 8 s-tiles
NI = D // 128          # 6 d-blocks
f32 = mybir.dt.float32
f16 = mybir.dt.float16
f32r = mybir.dt.float32r
# QEr scratch packing: strips si (width 128*(si+1)) stored in PAIRS at the
# pair's max width so a single 3D-AP DMA can read two strips at once.
PITCH = [128 * (2 * (si // 2) + 2) for si in range(NT)]   # per-strip pitch
G = [0]
for _g in range(NT // 2):
    G.append(G[-1] + 2 * PITCH[2 * _g])                   # group offsets
OFF2 = [G[si // 2] + (si % 2) * PITCH[si] for si in range(NT)]
RW = G[-1]             # 5120 scratch row width
# A^T packed-causal: t-block ti (s-width 1024-128*ti) at col OT[ti]
OT = [0]
for _ti in range(NT):
    OT.append(OT[-1] + S - 128 * _ti)
AW = OT[-1]            # 4608

HEAD_REG = 128 * RW          # per-head scratch region (elements)
PAIR_REG = 2 * HEAD_REG      # per-pair


def build_nc():
    nc = bacc.Bacc("TRN2", target_bir_lowering=False, debug=False,
                   num_devices=N_CORES)

    QT = nc.dram_tensor("QT", [D, S], f16, kind="ExternalInput")
    KT = nc.dram_tensor("KT", [D, S], f16, kind="ExternalInput")
    VT = nc.dram_tensor("VT", [D, S], f16, kind="ExternalInput")
    WqT = nc.dram_tensor("WqT", [D, D], f16, kind="ExternalInput")
    WkT = nc.dram_tensor("WkT", [D, D], f16, kind="ExternalInput")
    WvT = nc.dram_tensor("WvT", [D, D], f16, kind="ExternalInput")
    WoT = nc.dram_tensor("WoT", [D, D], f16, kind="ExternalInput")
    bq = nc.dram_tensor("bq", [D], f32, kind="ExternalInput")
    bk = nc.dram_tensor("bk", [D], f32, kind="ExternalInput")
    bv = nc.dram_tensor("bv", [D], f32, kind="ExternalInput")
    bo = nc.dram_tensor("bo", [D], f32, kind="ExternalInput")
    ErT = nc.dram_tensor("ErT", [DK, S], f16, kind="ExternalInput")
    out = nc.dram_tensor("out", [D, S], f32, kind="ExternalOutput")

    qer_dram = nc.dram_tensor("qer_scratch", [3 * PAIR_REG], f16)

    tensors = dict(QT=QT, KT=KT, VT=VT, WqT=WqT, WkT=WkT, WvT=WvT, WoT=WoT,
                   bq=bq, bk=bk, bv=bv, bo=bo, ErT=ErT, out=out,
                   qer_dram=qer_dram)
    unroll = int(os.environ.get("BASS_UNROLL", "1"))
    with tile.TileContext(nc) as tc:
        for _ in range(unroll):
            _build_body(nc, tc, tensors)
    nc.compile()
    return nc


def _load_blocked_half(nc, dst_tile, src_dram, nrow, ncol, nblk, halves, hh):
    hw = ncol // halves
    dst3 = dst_tile[:, :].rearrange(
        "p (b c) -> p b c", b=nblk)[:, :, hh * hw:(hh + 1) * hw]
    src = AP(tensor=src_dram, offset=hh * hw,
             ap=[[ncol, 128], [128 * ncol, nblk], [1, hw]])
    nc.sync.dma_start(out=dst3, in_=src)


def _load_blocked(nc, dst_tile, src_dram, nrow, ncol, nblk, halves=1):
    """src [nblk*128, ncol] -> dst [128, nblk*ncol] where dst block ib holds
    src rows [128*ib, 128*ib+128). `halves` splits along ncol for earlier
    consumption."""
    for hh in range(halves):
        _load_blocked_half(nc, dst_tile, src_dram, nrow, ncol, nblk, halves, hh)


def _build_body(nc, tc, t):
    QT, KT, VT = t["QT"], t["KT"], t["VT"]
    WqT, WkT, WvT, WoT = t["WqT"], t["WkT"], t["WvT"], t["WoT"]
    bq, bk, bv, bo = t["bq"], t["bk"], t["bv"], t["bo"]
    ErT, out, qer_dram = t["ErT"], t["out"], t["qer_dram"]

    from contextlib import ExitStack
    with ExitStack() as ctx:
        persist = ctx.enter_context(tc.tile_pool(name="persist", bufs=1))

        ident = persist.tile([128, 128], f32, tag="ident")
        make_identity(nc, ident[:])
        ident16 = persist.tile([128, 128], f16, tag="ident16")
        nc.vector.tensor_copy(ident16[:], ident[:])
        ident32r = persist.tile([128, 128], f32r, tag="ident32r")
        nc.vector.tensor_copy(ident32r[:], ident[:])
        ones16 = persist.tile([1, 64], f16, tag="ones16")
        nc.vector.memset(ones16[:], 1.0)
        # causal mask tile for diagonal blocks, [t, s]: -3e4 where t > s
        mask16 = persist.tile([128, 128], f16, tag="mask16")
        nc.vector.memset(mask16[:], 0.0)
        nc.gpsimd.affine_select(
            out=mask16[:], in_=mask16[:], pattern=[[1, 128]],
            compare_op=mybir.AluOpType.is_ge, fill=-30000.0, base=0,
            channel_multiplier=-1)

        # ---- biases (small, needed by first projection evictions) ----
        bq_col = persist.tile([128, NI], f32, tag="bq_col")
        nc.sync.dma_start(out=bq_col[:],
                          in_=AP(tensor=bq, offset=0, ap=[[1, 128], [128, NI]]))
        bk_col = persist.tile([128, NI], f32, tag="bk_col")
        nc.sync.dma_start(out=bk_col[:],
                          in_=AP(tensor=bk, offset=0, ap=[[1, 128], [128, NI]]))

        # ---- persistent tiles ----
        qT6 = [persist.tile([128, S], f16, tag=f"qT{i}", name=f"qT{i}")
               for i in range(NI)]
        kT6 = [persist.tile([128, S], f16, tag=f"kT{i}", name=f"kT{i}")
               for i in range(NI)]
        v16 = [persist.tile([128, H * 65], f16, tag=f"v16{i}", name=f"v16{i}")
               for i in range(NT)]
        woT = persist.tile([128, NI * D], f16, tag="woT")
        attn_outT = [persist.tile([128, S], f16, tag=f"aoT{i}", name=f"aoT{i}")
                     for i in range(NI)]
        erT2 = persist.tile([128, S], f16, tag="erT2")
        bv_row = persist.tile([128, D], f16, tag="bv_row")
        bo_col = persist.tile([128, NI], f32, tag="bo_col")

        # ---- projections + attention: software-pipelined emission ----
        with tc.tile_pool(name="stage_w", bufs=1) as wstg, \
             tc.tile_pool(name="stage_x", bufs=1) as xstg, \
             tc.tile_pool(name="ps_all", bufs=6, space="PSUM") as ps_all, \
             tc.tile_pool(name="work", bufs=2) as work, \
             tc.tile_pool(name="srelp", bufs=2) as srelp, \
             tc.tile_pool(name="ps_av", bufs=2, space="PSUM") as ps_av:
            # q^T / k^T : psum[j, s-chunk] = sum_ib WT[ib-block] @ XT[ib-block]
            def emit_qk_proj(X, W, bias_col, xT_out, wname):
                wtile = wstg.tile([128, NI * D], f16, tag="w", name=wname)
                xtile = xstg.tile([128, NI * S], f16, tag="x", name=f"x{wname}")
                # interleave halves so the first (w, x) pair lands ASAP
                for hh in range(2):
                    _load_blocked_half(nc, wtile, W, D, D, NI, 2, hh)
                    _load_blocked_half(nc, xtile, X, D, S, NI, 2, hh)
                if wname == "wq":
                    # ErT duplicated into both partition halves (needed by
                    # the first attention pair, right after the q projection)
                    nc.sync.dma_start(out=erT2[0:DK, :], in_=ErT.ap()[:, :])
                    nc.sync.dma_start(out=erT2[DK:128, :], in_=ErT.ap()[:, :])
                for sh in range(2):
                    for jt in range(NI):
                        p = ps_all.tile([128, 512], f32, tag="ps512")
                        for ib in range(NI):
                            nc.tensor.matmul(
                                p[:],
                                wtile[:, D * ib + 128 * jt:D * ib + 128 * jt + 128],
                                xtile[:, S * ib + 512 * sh:S * ib + 512 * sh + 512],
                                start=(ib == 0), stop=(ib == NI - 1),
                            )
                        with nc.allow_low_precision(reason="fp16 activations"):
                            nc.vector.tensor_scalar_add(
                                xT_out[jt][:, sh * 512:(sh + 1) * 512], p[:],
                                bias_col[:, jt:jt + 1],
                            )

            def emit_v_proj():
                # v natural: psum[t, j] = sum_ib XvT[ib][:, t-block] @ WvT[ib]
                nc.gpsimd.dma_start(
                    out=bv_row[:],
                    in_=AP(tensor=bv, offset=0, ap=[[0, 128], [1, D]]))
                for tt in range(NT):
                    ones_col = v16[tt][:, :].rearrange(
                        "p (a b) -> p a b", b=65)[:, :, 64:65]
                    nc.vector.memset(ones_col, 1.0)
                wtile = wstg.tile([128, NI * D], f16, tag="w", name="wv")
                _load_blocked(nc, wtile, WvT, D, D, NI, halves=2)
                xtile = xstg.tile([128, NI * S], f16, tag="x", name="xwv")
                _load_blocked(nc, xtile, VT, D, S, NI, halves=2)
                # deferred loads needed only by the output projection
                _load_blocked(nc, woT, WoT, D, D, NI, halves=2)
                nc.sync.dma_start(
                    out=bo_col[:],
                    in_=AP(tensor=bo, offset=0, ap=[[1, 128], [128, NI]]))
                for tt in range(NT):
                    for js, w in ((0, 512), (512, 256)):
                        p = ps_all.tile([128, 512], f32, tag="ps512")
                        for ib in range(NI):
                            nc.tensor.matmul(
                                p[:, :w],
                                xtile[:, S * ib + 128 * tt:S * ib + 128 * tt + 128],
                                wtile[:, D * ib + js:D * ib + js + w],
                                start=(ib == 0), stop=(ib == NI - 1),
                            )
                        hh0 = js // 64
                        nh = w // 64
                        dst3 = v16[tt][:, :].rearrange(
                            "p (a b) -> p a b", b=65)[:, hh0:hh0 + nh, 0:64]
                        with nc.allow_low_precision(reason="fp16 activations"):
                            nc.vector.tensor_add(
                                dst3, p[:, :w].rearrange("p (a b) -> p a b", b=64),
                                bv_row[:, js:js + w].rearrange("p (a b) -> p a b", b=64),
                            )

            pair_state = {}

            def stage_a(hp):
                # QEr strips -> fp16 -> DRAM scratch -> skewed read-back
                jb = hp
                pair_base = (hp % 3) * PAIR_REG
                ctxs = [dict(h=2 * hp + idx, jr=64 * idx, idx=idx)
                        for idx in range(2)]
                for c in ctxs:
                    c["srel_out"] = work.tile(
                        [128, RW], f16, tag="srel_out",
                        name=f"srel_out{c['h']}", bufs=2)
                    # pad columns of even strips hold -3e4: the diagonal
                    # tile's above-diagonal junk reads land exactly there,
                    # so the srel accumulate applies the causal mask for
                    # free (odd strips still need the mask matmul)
                    for si in range(0, NT, 2):
                        wv = 128 * (si + 1)
                        nc.vector.memset(
                            c["srel_out"][:, OFF2[si] + wv:OFF2[si] + PITCH[si]],
                            -30000.0)
                for si in range(NT):
                    Wcw = 128 * (si + 1)
                    e0 = S - Wcw
                    for cs in range(0, Wcw, 512):
                        w = min(512, Wcw - cs)
                        for c in ctxs:
                            jr = c["jr"]
                            qTs = qT6[jb][jr:jr + 64, si * 128:(si + 1) * 128]
                            p = ps_all.tile([128, 512], f32, tag="ps512")
                            nc.tensor.matmul(
                                p[:, :w], qTs,
                                erT2[jr:jr + 64, e0 + cs:e0 + cs + w],
                                start=True, stop=True)
                            dstp = c["srel_out"][:, OFF2[si] + cs:OFF2[si] + cs + w]
                            with nc.allow_low_precision(reason="fp16 srel"):
                                if (si + c["idx"]) % 2 == 0:
                                    nc.vector.tensor_copy(dstp, p[:, :w])
                                else:
                                    nc.scalar.copy(dstp, p[:, :w])
                for c in ctxs:
                    dst = AP(tensor=qer_dram,
                             offset=pair_base + c["idx"] * HEAD_REG,
                             ap=[[RW, 128], [1, RW]])
                    nc.sync.dma_start(out=dst, in_=c["srel_out"][:])

                # --- skew reads (strip-pair-batched, f16 -> f32r cast) ---
                for c in ctxs:
                    c["srel32"] = srelp.tile([128, RW], f32r, tag="srel32",
                                             name=f"srel32_{c['h']}", bufs=3)
                    head_base = pair_base + c["idx"] * HEAD_REG
                    for g in range(NT // 2):
                        pit = PITCH[2 * g]
                        skew = AP(tensor=qer_dram,
                                  offset=head_base + G[g] + 127,
                                  ap=[[RW - 1, 128], [pit, 2], [1, pit]])
                        dst3 = c["srel32"][:, :].rearrange(
                            "p (x) -> p x")[:, G[g]:G[g] + 2 * pit].rearrange(
                            "p (a b) -> p a b", b=pit)
                        nc.gpsimd.dma_start(out=dst3, in_=skew)
                pair_state[hp] = ctxs

            def stage_b(hp):
                # QK + srel-transpose-accumulate + causal mask + exp + AV
                jb = hp
                ctxs = pair_state.pop(hp)
                for c in ctxs:
                    c["A16T"] = work.tile([128, AW], f16, tag="A16T",
                                          name=f"A16T_{c['h']}", bufs=2)
                for ti in range(NT):
                    s0 = 128 * ti
                    w = S - s0
                    chunks = [(cs, min(512, w - cs)) for cs in range(0, w, 512)]
                    # all QK matmuls of this ti first: the two heads sit on
                    # disjoint PE row-halves, so their stationaries coexist
                    # and consecutive chunks reuse them (the full-width srel
                    # transpose stationaries would otherwise clobber them
                    # between chunks, forcing a kT reload per chunk)
                    pqks = {}
                    for cs, cw in chunks:
                        for c in ctxs:
                            jr = c["jr"]
                            pqk = ps_all.tile([128, 512], f32, tag="ps512",
                                              name=f"pqk{c['idx']}_{cs // 512}")
                            pqks[(c["idx"], cs)] = pqk
                            nc.tensor.matmul(
                                pqk[:, :cw],
                                kT6[jb][jr:jr + 64, ti * 128:(ti + 1) * 128],
                                qT6[jb][jr:jr + 64, s0 + cs:s0 + cs + cw],
                                start=True, stop=False)
                    for cs, cw in chunks:
                        for c in ctxs:
                            pqk = pqks[(c["idx"], cs)]
                            nch = cw // 128
                            has_mask = (cs == 0 and ti % 2 == 1)
                            for k in range(nch):
                                sic = ti + (cs + k * 128) // 128
                                last = (k == nch - 1) and not has_mask
                                nc.tensor.matmul(
                                    pqk[:, k * 128:(k + 1) * 128].bitcast(f32r),
                                    c["srel32"][:, OFF2[sic] + 128 * ti:
                                                OFF2[sic] + 128 * ti + 128],
                                    ident32r[:],
                                    is_transpose=True,
                                    start=False, stop=last)
                            if has_mask:
                                # causal mask on the diagonal block (odd
                                # strips have no pad region in the scratch)
                                nc.tensor.matmul(
                                    pqk[:, 0:128], ident16[:], mask16[:],
                                    start=False, stop=True)
                            nc.scalar.activation(
                                c["A16T"][:, OT[ti] + cs:OT[ti] + cs + cw],
                                pqk[:, :cw],
                                mybir.ActivationFunctionType.Exp, scale=0.125)

                # --- AV in two s-halves + normalize ---
                for c in ctxs:
                    h, jr, idx = c["h"], c["jr"], c["idx"]
                    for sh in range(2):
                        slo = 512 * sh
                        pav = ps_av.tile([65, 512], f32, tag="av",
                                         name=f"pav{idx}_{sh}")
                        tis = [ti for ti in range(NT) if 128 * ti < slo + 512]
                        for ti in tis:
                            lo = max(slo, 128 * ti)
                            a0 = OT[ti] + lo - 128 * ti
                            nc.tensor.matmul(
                                pav[:, lo - slo:512],
                                v16[ti][:, h * 65:(h + 1) * 65],
                                c["A16T"][:, a0:a0 + (slo + 512 - lo)],
                                start=(ti == tis[0]), stop=(ti == tis[-1]))
                        rZ = work.tile([1, 512], f16, tag="rZ",
                                       name=f"rZ{h}_{sh}")
                        with nc.allow_low_precision(reason="fp16 softmax Z"):
                            nc.vector.reciprocal(rZ[:], pav[64:65, :])
                        prz = ps_all.tile([64, 512], f32, tag="ps512",
                                         name=f"prz{idx}")
                        nc.tensor.matmul(prz[:], ones16[:], rZ[:],
                                         start=True, stop=True)
                        rzb = work.tile([64, 512], f16, tag="rzb",
                                        name=f"rzb{idx}_{sh}", bufs=2)
                        with nc.allow_low_precision(reason="fp16 attn out"):
                            if (idx + sh) % 2 == 0:
                                nc.vector.tensor_copy(rzb[:], prz[:])
                            else:
                                nc.scalar.copy(rzb[:], prz[:])
                            if idx == 0:
                                nc.vector.tensor_mul(
                                    attn_outT[jb][0:64, slo:slo + 512],
                                    pav[0:64, :], rzb[:])
                            else:
                                odd_tmp = work.tile([64, 512], f16,
                                                    tag="odd_tmp",
                                                    name=f"ot{h}_{sh}", bufs=2)
                                nc.vector.tensor_mul(
                                    odd_tmp[:], pav[0:64, :], rzb[:])
                                nc.sync.dma_start(
                                    out=attn_outT[jb][64:128, slo:slo + 512],
                                    in_=odd_tmp[:])

            # --- pipelined emission: stage A runs two pairs ahead ---
            NP = H // 2
            emit_qk_proj(QT, WqT, bq_col, qT6, "wq")
            stage_a(0)
            emit_qk_proj(KT, WkT, bk_col, kT6, "wk")
            stage_a(1)
            emit_v_proj()
            for hp in range(NP):
                stage_b(hp)
                if hp + 2 < NP:
                    stage_a(hp + 2)

            # ---- output projection (stored transposed; host un-transposes)
            # reuses the attention pools so it can interleave with the
            # final pairs instead of waiting for all PSUM banks to free
            for sh in range(2):
                for jt in range(NI):
                    p = ps_all.tile([128, 512], f32, tag="ps512", name="po")
                    for ib in range(NI):
                        nc.tensor.matmul(
                            p[:],
                            woT[:, D * ib + 128 * jt:D * ib + 128 * jt + 128],
                            attn_outT[ib][:, sh * 512:(sh + 1) * 512],
                            start=(ib == 0), stop=(ib == NI - 1))
                    osb = work.tile([128, 512], f32, tag="osb", bufs=2)
                    nc.vector.tensor_scalar_add(osb[:], p[:],
                                                bo_col[:, jt:jt + 1])
                    nc.sync.dma_start(
                        out=out.ap()[jt * 128:(jt + 1) * 128,
                                     sh * 512:(sh + 1) * 512],
                        in_=osb[:])


_NC = None


def make_in_maps(**inputs):
    f = np.float16
    Q = np.asarray(inputs["Q"], dtype=np.float32)
    K = np.asarray(inputs["K"], dtype=np.float32)
    V = np.asarray(inputs["V"], dtype=np.float32)
    shared = {
        "WqT": np.ascontiguousarray(np.asarray(inputs["Wq"]).T.astype(f)),
        "WkT": np.ascontiguousarray(np.asarray(inputs["Wk"]).T.astype(f)),
        "WvT": np.ascontiguousarray(np.asarray(inputs["Wv"]).T.astype(f)),
        "WoT": np.ascontiguousarray(np.asarray(inputs["Wo"]).T.astype(f)),
        "ErT": np.ascontiguousarray(np.asarray(inputs["Er"]).T.astype(f)),
        "bq": np.ascontiguousarray(np.asarray(inputs["bq"], dtype=np.float32)),
        "bk": np.ascontiguousarray(np.asarray(inputs["bk"], dtype=np.float32)),
        "bv": np.ascontiguousarray(np.asarray(inputs["bv"], dtype=np.float32)),
        "bo": np.ascontiguousarray(np.asarray(inputs["bo"], dtype=np.float32)),
    }
    return [
        {
            "QT": np.ascontiguousarray(Q[c].T.astype(f)),
            "KT": np.ascontiguousarray(K[c].T.astype(f)),
            "VT": np.ascontiguousarray(V[c].T.astype(f)),
            **shared,
        }
        for c in range(N_CORES)
    ]


def unshard(shards):
    # kernel stores out^T [D, S]; un-transpose host-side
    return np.stack([np.ascontiguousarray(shards[c].T) for c in range(N_CORES)],
                    axis=0)


def kernel(**inputs):
    global _NC
    if _NC is None:
        _NC = build_nc()
    in_maps = make_in_maps(**inputs)
    global _last_in_maps
    _last_in_maps = in_maps
    res = run_bass_kernel_spmd(_NC, in_maps, list(range(N_CORES)))
    return unshard([res.results[c]["out"] for c in range(N_CORES)])
